# revision 1
# baseline (speedup 1.0000x reference)
"""Trainium2 Bass kernel for nn_BoundaryLoss: mean(|softmax(pred) * SDF(onehot(target))|).

Strategy (8 NeuronCores, SPMD):
  - One (b, c) pair per core (B=2 x C=4 = 8 pairs). Each core computes the exact
    3D squared Euclidean distance transform of the class-c seed mask (pos) and its
    complement (neg) for its batch element, via separable truncated-shift min-plus
    passes (shift radius S derived from the input on the host; truncation at
    S >= max true distance is exact). |sdf| = sqrt(g_pos + g_neg) since exactly one
    of the two is zero at every voxel. The core then multiplies by softmax(pred)[c]
    and reduces to 48 per-partition partial sums.
  - Host shards inputs, sums the 8x48 partials, applies the has_pos gate and the
    1/(B*C*D*H*W) mean factor.

Layout per core: SBUF tiles [NP, 2304] with partition rows
  [0,S): INF border | [S, S+48): pos volume (row S+d) | [S+48, 2S+48): INF gap |
  [2S+48, 2S+96): neg volume | [2S+96, 3S+96): INF border
free dim = (h, w) flattened. W/H passes shift along free dims; the D pass uses
partition-offset SBUF->SBUF DMA copies (compute ops never straddle partitions).
EDT arithmetic is int16 (exact: all squared distances are integers <= 6627; INF
is 30000 and never overflows: 30000 + 3*47^2 < 32767).
"""

import os
import sys

import numpy as np

B, C, DD, HH, WW = 2, 4, 48, 48, 48
PLANE = HH * WW  # free size 2304
NVOX = DD * PLANE
INF16 = 30000.0
S_MAX = 16  # gap/tail rows bound the shift radius
N_CORES = 8

_nc_cache = {}
LAST_RESULTS = None  # test harness introspection


def _ensure_paths():
    for p in ("/opt/trn_rl_repo",):
        if os.path.isdir(p) and p not in sys.path:
            sys.path.insert(0, p)


def _edt_sq_trunc_np(f0, S):
    """Truncated-shift separable squared EDT (numpy, int32). Mirrors the device
    algorithm; used for the shift-bound certification and the fallback path."""
    f = f0.astype(np.int32)
    for ax in (2, 1, 0):
        g = f.copy()
        for s in range(1, S + 1):
            s2 = s * s
            sl_out = [slice(None)] * 3
            sl_in = [slice(None)] * 3
            sl_out[ax] = slice(s, None)
            sl_in[ax] = slice(None, -s)
            np.minimum(g[tuple(sl_out)], f[tuple(sl_in)] + s2, out=g[tuple(sl_out)])
            sl_out[ax] = slice(None, -s)
            sl_in[ax] = slice(s, None)
            np.minimum(g[tuple(sl_out)], f[tuple(sl_in)] + s2, out=g[tuple(sl_out)])
        f = g
    return f


def _certified_shift_bound(masks):
    """Smallest S such that the S-truncated separable EDT is provably exact for
    every seed mask in `masks`: if the truncated result's max distance is <= S,
    truncation never cut off a winning chain (truncation only overestimates, so
    max_true <= max_trunc <= S certifies S >= max per-axis seed offset)."""
    for S in range(1, S_MAX + 1):
        worst = 0
        for m in masks:
            f0 = np.where(m, 0, 30000).astype(np.int16)
            g = _edt_sq_trunc_np(f0, S)
            worst = max(worst, int(np.ceil(np.sqrt(float(g.max())))))
        if worst <= S:
            return S
    return S_MAX + 1  # triggers the fallback path


def _reference_fallback(pred, target):
    """Exact numpy replica of the reference for pathological inputs the device
    path does not cover (wrong shapes, empty masks, S > S_MAX)."""
    INF = 1e9
    pred = np.asarray(pred, np.float32)
    target = np.asarray(target)
    b_, c_ = pred.shape[0], pred.shape[1]
    n = np.arange(pred.shape[-1])

    def minplus(f):
        d2 = ((n[:, None] - n[None, :]) ** 2).astype(np.float32)
        return (f[..., None, :] + d2).min(axis=-1)

    def edt(src):
        f = np.where(src, 0.0, INF).astype(np.float32)
        for ax in (-3, -2, -1):
            f = np.moveaxis(minplus(np.moveaxis(f, ax, -1)), -1, ax)
        return np.sqrt(f)

    e = np.exp(pred - pred.max(axis=1, keepdims=True))
    sm = e / e.sum(axis=1, keepdims=True)
    total = 0.0
    for b in range(b_):
        for c in range(c_):
            pos = target[b] == c
            if not pos.any():
                continue
            sdf = edt(pos) - edt(~pos)
            total += float(np.abs(sm[b, c] * sdf).sum(dtype=np.float64))
    return np.float32(total / pred.size)


def _build_nc(S):
    """Build + compile the SPMD Bass program for shift radius S.

    Row layout (128 partitions; compute partition ranges must start naturally
    aligned: count<=32 -> 32-aligned start, <=64 -> 64-aligned, >64 -> start 0):
      [0,48) pos volume | [48,64) INF gap | [64,112) neg volume | [112,128) INF
    """
    _ensure_paths()
    import concourse.tile as tile
    from concourse import bacc, mybir

    i16 = mybir.dt.int16
    f32 = mybir.dt.float32
    ALU = mybir.AluOpType
    ACT = mybir.ActivationFunctionType

    NP = 128
    RB = 64            # neg block start row
    RV = 112           # end of valid rows (compute range [0, RV))

    nc = bacc.Bacc("TRN2", target_bir_lowering=False, debug=False)

    tgt_d = nc.dram_tensor("tgt", [NP, PLANE], i16, kind="ExternalInput")
    cv_d = nc.dram_tensor("cvec", [NP, 1], f32, kind="ExternalInput")
    pred_d = nc.dram_tensor("pred4", [C, DD, PLANE], f32, kind="ExternalInput")
    pm_d = nc.dram_tensor("pairmat", [NP, 48], f32, kind="ExternalInput")
    out_d = nc.dram_tensor("out", [48, 1], f32, kind="ExternalOutput")

    with tile.TileContext(nc) as tc:
        with (
            tc.tile_pool(name="main", bufs=1) as pool,
            tc.tile_pool(name="fsp", bufs=4) as fsp,
            tc.tile_pool(name="psum", bufs=1, space="PSUM") as psp,
        ):
            Tt = pool.tile([NP, PLANE], i16, tag="T")
            nc.sync.dma_start(Tt[:], tgt_d[:])
            CV = pool.tile([NP, 1], f32, tag="cv")
            nc.sync.dma_start(CV[:], cv_d[:])
            PM = pool.tile([NP, 48], f32, tag="pm")
            nc.sync.dma_start(PM[:], pm_d[:])
            PR = pool.tile([48, C * PLANE], f32, tag="pr")
            nc.sync.dma_start(PR[:], pred_d.rearrange("c p n -> p c n"))

            A = pool.tile([NP, PLANE], i16, tag="A")
            Bt = pool.tile([NP, PLANE], i16, tag="B")

            # onehot init: pos rows f = (t != c)*INF, neg rows f = (t == c)*INF.
            # Host sentinel rows make the gap come out INF; tail memset to INF.
            nc.gpsimd.memset(A[96:NP, :], INF16)
            nc.vector.tensor_scalar(
                out=A[0:RB, :], in0=Tt[0:RB, :], scalar1=CV[0:RB, :],
                scalar2=INF16, op0=ALU.not_equal, op1=ALU.mult,
            )
            nc.vector.tensor_scalar(
                out=A[RB:RV, :], in0=Tt[RB:RV, :], scalar1=CV[RB:RV, :],
                scalar2=INF16, op0=ALU.is_equal, op1=ALU.mult,
            )

            def freepass(src, dst, axis_w):
                """min-plus pass along w (axis_w=True) or h (False), src -> dst."""
                s3 = src[:].rearrange("p (h w) -> p h w", w=WW)
                d3 = dst[:].rearrange("p (h w) -> p h w", w=WW)
                nc.vector.tensor_copy(dst[0:RV, :], src[0:RV, :])
                for s in range(1, S + 1):
                    s2 = float(s * s)
                    if axis_w:
                        pairs = [
                            (d3[0:RV, :, s:], s3[0:RV, :, : WW - s]),
                            (d3[0:RV, :, : WW - s], s3[0:RV, :, s:]),
                        ]
                    else:
                        pairs = [
                            (d3[0:RV, s:, :], s3[0:RV, : HH - s, :]),
                            (d3[0:RV, : HH - s, :], s3[0:RV, s:, :]),
                        ]
                    for dap, sap in pairs:
                        nc.vector.scalar_tensor_tensor(
                            out=dap, in0=sap, scalar=s2, in1=dap,
                            op0=ALU.add, op1=ALU.min,
                        )

            freepass(A, Bt, axis_w=True)   # pass along W
            freepass(Bt, A, axis_w=False)  # pass along H

            # pass along D: partition-offset DMA copies + aligned STT updates.
            # A's gap/tail rows are INF so shifted reads never leak across blocks.
            nc.vector.tensor_copy(Bt[0:RV, :], A[0:RV, :])
            for s in range(1, S + 1):
                s2 = float(s * s)
                for sign in (1, -1):
                    fs = fsp.tile([NP, PLANE], i16, tag="fs")
                    if sign > 0:
                        nc.gpsimd.memset(fs[0:32, :], INF16)
                        nc.sync.dma_start(fs[s:NP, :], A[0 : NP - s, :])
                    else:
                        nc.gpsimd.memset(fs[96:NP, :], INF16)
                        nc.sync.dma_start(fs[0 : NP - s, :], A[s:NP, :])
                    nc.vector.scalar_tensor_tensor(
                        out=Bt[0:RV, :], in0=fs[0:RV, :], scalar=s2,
                        in1=Bt[0:RV, :], op0=ALU.add, op1=ALU.min,
                    )

            # |sdf| = sqrt(g_pos + g_neg): sqrt rows, then pair-sum via PE matmul
            SQ = pool.tile([NP, PLANE], f32, tag="SQ")
            nc.gpsimd.memset(SQ[96:NP, :], 0.0)
            nc.scalar.activation(SQ[0:RV, :], Bt[0:RV, :], ACT.Sqrt)
            PS = psp.tile([48, PLANE], f32, tag="ps")
            n0 = 0
            while n0 < PLANE:
                nn = min(512, PLANE - n0)
                nc.tensor.matmul(
                    PS[:, n0 : n0 + nn], PM[:], SQ[:, n0 : n0 + nn],
                    start=True, stop=True,
                )
                n0 += nn

            # softmax weight for class c (host permuted class c to slot 0)
            nc.scalar.activation(PR[:], PR[:], ACT.Exp)
            DN = pool.tile([48, PLANE], f32, tag="dn")
            nc.vector.tensor_tensor(DN[:], PR[:, 0:PLANE], PR[:, PLANE : 2 * PLANE], ALU.add)
            nc.vector.tensor_tensor(DN[:], DN[:], PR[:, 2 * PLANE : 3 * PLANE], ALU.add)
            nc.vector.tensor_tensor(DN[:], DN[:], PR[:, 3 * PLANE : 4 * PLANE], ALU.add)
            RC = pool.tile([48, PLANE], f32, tag="rc")
            nc.vector.reciprocal(RC[:], DN[:])
            nc.vector.tensor_tensor(DN[:], PR[:, 0:PLANE], RC[:], ALU.mult)

            # partial[d] = sum_(h,w) |sdf| * w_c
            AC = pool.tile([48, 1], f32, tag="ac")
            nc.vector.tensor_tensor(SQ[0:48, :], PS[:], DN[:], ALU.mult)
            nc.vector.reduce_sum(AC[:], SQ[0:48, :], axis=mybir.AxisListType.X)
            nc.sync.dma_start(out_d[:], AC[:])

    nc.compile()
    return nc


def kernel(pred, target):
    pred = np.ascontiguousarray(np.asarray(pred), dtype=np.float32)
    target = np.asarray(target)

    if pred.shape != (B, C, DD, HH, WW) or target.shape != (B, DD, HH, WW):
        return _reference_fallback(pred, target)

    tgt = target.astype(np.int64)
    masks = []
    has_pos = {}
    for b in range(B):
        for c in range(C):
            m = tgt[b] == c
            has_pos[(b, c)] = bool(m.any())
            if has_pos[(b, c)]:
                masks.append(m)
                mn = ~m
                if mn.any():
                    masks.append(mn)
                else:
                    return _reference_fallback(pred, target)  # class fills volume

    S = _certified_shift_bound(masks)
    if S > S_MAX:
        return _reference_fallback(pred, target)

    _ensure_paths()
    from concourse.bass_utils import run_bass_kernel_spmd

    if S not in _nc_cache:
        _nc_cache[S] = _build_nc(S)
    nc = _nc_cache[S]

    NP, RB = 128, 64

    pairmat = np.zeros((NP, 48), np.float32)
    pairmat[np.arange(48), np.arange(48)] = 1.0
    pairmat[RB + np.arange(48), np.arange(48)] = 1.0

    in_maps = []
    for k in range(N_CORES):
        b, c = divmod(k, C)
        t16 = tgt[b].reshape(DD, PLANE).astype(np.int16)
        T = np.empty((NP, PLANE), np.int16)
        T[0:48] = t16
        T[48:RB] = 5        # gap rows: != c -> INF
        T[RB : RB + 48] = t16
        T[RB + 48 :] = c    # unused tail rows
        cvec = np.full((NP, 1), c, np.float32)
        perm = [c] + [j for j in range(C) if j != c]
        pred4 = np.ascontiguousarray(pred[b][perm].reshape(C, DD, PLANE))
        in_maps.append({"tgt": T, "cvec": cvec, "pred4": pred4, "pairmat": pairmat})

    trace = bool(os.environ.get("BOUNDARY_KERNEL_TRACE"))
    if trace:
        import importlib.util

        if importlib.util.find_spec("antenv.axon_hooks") is None:
            trace = False  # NTFF hook unavailable in this axon build
    res = run_bass_kernel_spmd(nc, in_maps, list(range(N_CORES)), trace=trace)
    global LAST_RESULTS
    LAST_RESULTS = res

    total = 0.0
    for k in range(N_CORES):
        b, c = divmod(k, C)
        if has_pos[(b, c)]:
            total += float(res.results[k]["out"].astype(np.float64).sum())
    return np.float32(total / (B * C * NVOX))


if __name__ == "__main__":
    import reference

    inputs = reference.setup_inputs()
    out = kernel(**{k: np.asarray(v) for k, v in inputs.items()})
    print("kernel out:", out)

